# revision 3
# baseline (speedup 1.0000x reference)
"""Gumbel Top-K gate kernel for Trainium2 (8 NeuronCores, SPMD).

Math: mask[b, 0, r, m] = 1 iff z[b, r, m] is among the top-16 of row r, where
  z = mean_h(q_h k_h^T)/sqrt(64) + gumbel(u),  gumbel = -log(-log(u+eps)+eps).
Softmax is strictly monotone per row, so the reference's softmax/top-k mask
equals thresholding z at its 16th-largest value per row (ties included via >=).

Sharding: core c handles batch b = c//2, row half c%2 -> [1024, 2048] slab.
Head-mean folds into one [1024, 512] x [512, 2048] matmul per core
(concat heads along the contraction dim; 1/64 = scale * mean absorbed into Q).
"""

import sys

sys.path.insert(0, "/opt/trn_rl_repo")

import numpy as np

import concourse.bacc as bacc
import concourse.mybir as mybir
import concourse.tile as tile
from concourse import bass_utils

B, H, N, D = 4, 8, 2048, 64
HD = H * D  # 512 contraction dim (heads concatenated)
N_CORES = 8
ROWS = N * B // N_CORES  # 1024 rows per core
P = 128
EPS = 1e-9
NEG_BIG = -3.0e38
F32 = mybir.dt.float32


def _make_identity(nc, ident, fill):
    nc.gpsimd.memset(ident, 0.0)
    sq = ident.shape[0]
    nc.gpsimd.affine_select(
        out=ident,
        in_=ident,
        compare_op=mybir.AluOpType.not_equal,
        fill=fill,
        base=0,
        pattern=[[-1, sq]],
        channel_multiplier=1,
    )


def _build_body(tc, qc, kc, u_d, mask_d):
    nc = tc.nc
    n_rtiles = ROWS // P  # 8
    n_c = HD // P  # 4 contraction chunks
    act = mybir.ActivationFunctionType

    with (
        tc.tile_pool(name="consts", bufs=1) as consts,
        tc.tile_pool(name="kqT", bufs=1) as kqT_pool,
    ):
        ident = consts.tile([P, P], F32)
        _make_identity(nc, ident, 1.0)
        nident = consts.tile([P, P], F32)
        _make_identity(nc, nident, -1.0)
        eps_tile = consts.tile([P, 1], F32)
        nc.vector.memset(eps_tile, EPS)

        kT = kqT_pool.tile([P, n_c, N], F32)  # K^T: [128d, 4c, 2048m]
        qT = kqT_pool.tile([P, n_c, ROWS], F32)  # Q^T/64: [128d, 4c, 1024r]

        # Phase A: load q/k natural, PE-transpose into d-major layout.
        with (
            tc.tile_pool(name="nat", bufs=2) as nat_pool,
            tc.tile_pool(name="tp_psum", bufs=2, space="PSUM") as tp_psum,
        ):
            kc_g = kc.rearrange("(g q p) d -> g p q d", q=4, p=P)
            for g in range(4):
                natk = nat_pool.tile([P, 4, HD], F32, tag="nat")
                nc.sync.dma_start(out=natk, in_=kc_g[g])
                for c in range(n_c):
                    pt = tp_psum.tile([P, 512], F32, tag="tp")
                    for q in range(4):
                        nc.tensor.transpose(
                            pt[:, q * P : (q + 1) * P],
                            natk[:, q, c * P : (c + 1) * P],
                            ident,
                        )
                    nc.scalar.activation(
                        kT[:, c, g * 512 : (g + 1) * 512], pt, act.Copy
                    )
            qc_g = qc.rearrange("(g q p) d -> g p q d", q=4, p=P)
            for g in range(2):
                natq = nat_pool.tile([P, 4, HD], F32, tag="nat")
                nc.sync.dma_start(out=natq, in_=qc_g[g])
                for c in range(n_c):
                    pt = tp_psum.tile([P, 512], F32, tag="tp")
                    for q in range(4):
                        nc.tensor.transpose(
                            pt[:, q * P : (q + 1) * P],
                            natq[:, q, c * P : (c + 1) * P],
                            ident,
                        )
                    # 1/64 = (1/sqrt(64)) * (1/8 head mean), folded into Q
                    nc.scalar.activation(
                        qT[:, c, g * 512 : (g + 1) * 512], pt, act.Copy, scale=1.0 / 64
                    )

        # Phase B: per 128-row tile: gumbel, scores, top-16 threshold, mask.
        u_t = u_d.rearrange("(t p) n -> t p n", p=P)
        mask_t = mask_d.rearrange("(t p) n -> t p n", p=P)
        with (
            tc.tile_pool(name="s_psum", bufs=2, space="PSUM") as s_psum,
            tc.tile_pool(name="work", bufs=2) as work,
            tc.tile_pool(name="uin", bufs=3) as uin,
            tc.tile_pool(name="mout", bufs=2) as mout,
            tc.tile_pool(name="small", bufs=2) as small,
        ):
            for t in range(n_rtiles):
                ut = uin.tile([P, N], F32, tag="u")
                nc.sync.dma_start(out=ut, in_=u_t[t])
                g1 = work.tile([P, N], F32, tag="g1")
                nc.scalar.activation(g1, ut, act.Ln, bias=eps_tile, scale=1.0)
                # g2 = log(-log(u+eps)+eps); z = S - g2
                g2 = work.tile([P, N], F32, tag="g2")
                nc.scalar.activation(g2, g1, act.Ln, bias=eps_tile, scale=-1.0)

                S = s_psum.tile([P, N], F32, tag="S")  # 4 PSUM banks
                for c in range(n_c):
                    for m in range(4):
                        nc.tensor.matmul(
                            S[:, m * 512 : (m + 1) * 512],
                            qT[:, c, t * P : (t + 1) * P],
                            kT[:, c, m * 512 : (m + 1) * 512],
                            start=(c == 0),
                            stop=False,
                        )
                for m in range(4):
                    # accumulate -g2 into PSUM via negated-identity matmul
                    nc.tensor.matmul(
                        S[:, m * 512 : (m + 1) * 512],
                        nident,
                        g2[:, m * 512 : (m + 1) * 512],
                        start=False,
                        stop=True,
                    )

                z = work.tile([P, N], F32, tag="z")
                nc.scalar.activation(z, S, act.Copy)  # PSUM -> SBUF on ACT

                m8a = small.tile([P, 8], F32, tag="m8a")
                nc.vector.max(out=m8a, in_=z)
                zs = work.tile([P, N], F32, tag="zs")
                nc.vector.match_replace(
                    out=zs, in_to_replace=m8a, in_values=z, imm_value=NEG_BIG
                )
                m8b = small.tile([P, 8], F32, tag="m8b")
                nc.vector.max(out=m8b, in_=zs)

                mk = mout.tile([P, N], F32, tag="mk")
                nc.vector.tensor_scalar(
                    out=mk,
                    in0=z,
                    scalar1=m8b[:, 7:8],
                    scalar2=None,
                    op0=mybir.AluOpType.is_ge,
                )
                nc.sync.dma_start(out=mask_t[t], in_=mk)


def build_kernel():
    nc = bacc.Bacc(
        "TRN2", target_bir_lowering=False, debug=False, num_devices=N_CORES
    )
    qc = nc.dram_tensor("qc", [ROWS, HD], F32, kind="ExternalInput").ap()
    kc = nc.dram_tensor("kc", [N, HD], F32, kind="ExternalInput").ap()
    u = nc.dram_tensor("u", [ROWS, N], F32, kind="ExternalInput").ap()
    mask = nc.dram_tensor("mask", [ROWS, N], F32, kind="ExternalOutput").ap()
    with tile.TileContext(nc) as tc:
        _build_body(tc, qc, kc, u, mask)
    nc.compile()
    return nc


_NC_CACHE = None
LAST_RESULTS = None


def _get_nc():
    global _NC_CACHE
    if _NC_CACHE is None:
        _NC_CACHE = build_kernel()
    return _NC_CACHE


def make_in_maps(q, k, u):
    q = np.asarray(q, np.float32)
    k = np.asarray(k, np.float32)
    u = np.asarray(u, np.float32)
    in_maps = []
    kc_by_batch = {}
    for core in range(N_CORES):
        b, half = divmod(core, 2)
        r0 = half * ROWS
        if b not in kc_by_batch:
            kc_by_batch[b] = np.ascontiguousarray(
                k[b].transpose(1, 0, 2).reshape(N, HD)
            )
        qc = np.ascontiguousarray(
            q[b, :, r0 : r0 + ROWS, :].transpose(1, 0, 2).reshape(ROWS, HD)
        )
        in_maps.append(
            {
                "qc": qc,
                "kc": kc_by_batch[b],
                "u": np.ascontiguousarray(u[b, r0 : r0 + ROWS]),
            }
        )
    return in_maps


def kernel(q, k, u):
    global LAST_RESULTS
    in_maps = make_in_maps(q, k, u)
    res = bass_utils.run_bass_kernel_spmd(
        _get_nc(), in_maps, core_ids=list(range(N_CORES))
    )
    LAST_RESULTS = res
    out = np.empty((B, 1, N, N), np.float32)
    for core in range(N_CORES):
        b, half = divmod(core, 2)
        r0 = half * ROWS
        out[b, 0, r0 : r0 + ROWS] = res.results[core]["mask"]
    return out
